# revision 1
# baseline (speedup 1.0000x reference)
"""Trainium2 Bass kernel for nn_AttentionTorch_77833397338547.

Computation (per batch b):
  K = keys[b,:,0,:]      [C=2048, S=1024]   (C = 16 heads x 128 head_dim)
  per head h (rows h*128:(h+1)*128 of the channel dim):
    scores[k, q] = (1/sqrt(128)) * K_h^T @ Q_h          [1024, 1024]
    P = softmax_k(scores + mask_bias)
    hid_h[d, q]  = V_h @ P                              [128, 1024]
  out[o, q] = sum_c w_out[o, c] * hid[c, q]             [2048, 1024]

Sharding: 8 cores = (batch b in 0..3) x (query half qh in 0..1).
Each core computes the full attention + out_proj for its (b, q-slice).
No cross-core communication is needed because out_proj only mixes
channels, which stay local to a core.

All matmuls run as float32r (single-pass fp32 on the PE, ~1.3e-4 rel
accuracy, 4x the throughput of plain fp32).
"""

import sys

sys.path.insert(0, "/opt/trn_rl_repo")

import numpy as np

B, C, S = 4, 2048, 1024
H, D = 16, 128          # heads x head_dim
QB = S // 2             # per-core query block = 512
KC = S // D             # key chunks per head = 8
OC = C // D             # out_proj row chunks = 16
N_CORES = 8
SCALE = 1.0 / np.sqrt(np.float32(D))
MASK_BIAS = np.float32(-60.0)

_BUILT = {}


def build_nc(repeat: int = 1):
    """Build + compile the per-core Bass program. Cached per repeat count."""
    if repeat in _BUILT:
        return _BUILT[repeat]

    import concourse.bass as bass
    import concourse.mybir as mybir
    import concourse.tile as tile
    from concourse import bacc

    f32 = mybir.dt.float32
    f32r = mybir.dt.float32r
    EXP = mybir.ActivationFunctionType.Exp

    nc = bacc.Bacc("TRN2", target_bir_lowering=False, debug=False,
                   num_devices=N_CORES)

    k_d = nc.dram_tensor("k_in", [C, S], f32r, kind="ExternalInput")
    q_d = nc.dram_tensor("q_in", [C, QB], f32r, kind="ExternalInput")
    v_d = nc.dram_tensor("v_in", [H, D, KC, D], f32r, kind="ExternalInput")
    w_d = nc.dram_tensor("w_in", [OC, D, H, D], f32r, kind="ExternalInput")
    bias_d = nc.dram_tensor("bias_in", [D, KC], f32, kind="ExternalInput")
    ones_d = nc.dram_tensor("ones_in", [D, D], f32r, kind="ExternalInput")
    out_d = nc.dram_tensor("out", [C, QB], f32, kind="ExternalOutput")

    def body(tc):
        with (
            tc.tile_pool(name="const", bufs=1) as const,
            tc.tile_pool(name="kvq", bufs=2) as kvq,
            tc.tile_pool(name="ep", bufs=2) as ep,
            tc.tile_pool(name="hidp", bufs=1) as hidp,
            tc.tile_pool(name="wp", bufs=3) as wp,
            tc.tile_pool(name="rcp", bufs=2) as rcp,
            tc.tile_pool(name="osb", bufs=3) as osb,
            tc.tile_pool(name="scp", bufs=3, space="PSUM") as scp,
            tc.tile_pool(name="hpp", bufs=2, space="PSUM") as hpp,
            tc.tile_pool(name="dnp", bufs=1, space="PSUM") as dnp,
            tc.tile_pool(name="opp", bufs=2, space="PSUM") as opp,
        ):
            ones_sb = const.tile([D, D], f32r)
            bias_sb = const.tile([D, KC], f32)
            nc.sync.dma_start(ones_sb[:], ones_d[:])
            nc.sync.dma_start(bias_sb[:], bias_d[:])

            hid_all = hidp.tile([D, H, QB], f32r)

            for h in range(H):
                k_sb = kvq.tile([D, S], f32r)
                q_sb = kvq.tile([D, QB], f32r)
                v_sb = kvq.tile([D, KC, D], f32r)
                nc.sync.dma_start(k_sb[:], k_d[h * D:(h + 1) * D, :])
                nc.sync.dma_start(q_sb[:], q_d[h * D:(h + 1) * D, :])
                nc.sync.dma_start(v_sb[:], v_d[h])

                e_sb = ep.tile([D, KC, QB], f32r)
                for c in range(KC):
                    sc = scp.tile([D, QB], f32)
                    nc.tensor.matmul(sc[:], k_sb[:, c * D:(c + 1) * D], q_sb[:],
                                     start=True, stop=True)
                    nc.scalar.activation(e_sb[:, c, :], sc[:], EXP,
                                         bias=bias_sb[:, c:c + 1], scale=1.0)

                dn = dnp.tile([D, QB], f32)
                for c in range(KC):
                    nc.tensor.matmul(dn[:], ones_sb[:], e_sb[:, c, :],
                                     start=(c == 0), stop=(c == KC - 1))
                hp = hpp.tile([D, QB], f32)
                for c in range(KC):
                    nc.tensor.matmul(hp[:], v_sb[:, c, :], e_sb[:, c, :],
                                     start=(c == 0), stop=(c == KC - 1))

                rc = rcp.tile([D, QB], f32)
                nc.vector.reciprocal(rc[:], dn[:])
                nc.vector.tensor_mul(hid_all[:, h, :], hp[:], rc[:])

            for j in range(OC):
                w_sb = wp.tile([D, H, D], f32r)
                nc.sync.dma_start(w_sb[:], w_d[j])
                op = opp.tile([D, QB], f32)
                for cc in range(H):
                    nc.tensor.matmul(op[:], w_sb[:, cc, :], hid_all[:, cc, :],
                                     start=(cc == 0), stop=(cc == H - 1))
                o_sb = osb.tile([D, QB], f32)
                nc.vector.tensor_copy(o_sb[:], op[:])
                nc.sync.dma_start(out_d[j * D:(j + 1) * D, :], o_sb[:])

    with tile.TileContext(nc) as tc:
        if repeat == 1:
            body(tc)
        else:
            PE = mybir.EngineType.PE
            ACT = mybir.EngineType.Activation
            DVE = mybir.EngineType.DVE
            SP = mybir.EngineType.SP
            with tc.For_i(0, repeat, 1, hint_engines=(PE, ACT, DVE, SP)):
                body(tc)

    nc.compile()
    _BUILT[repeat] = nc
    return nc


def shard_inputs(keys, values, queries, attention_mask, w_out):
    """Host-side prep: slice per core and pre-layout for the device."""
    keys = np.ascontiguousarray(np.asarray(keys, dtype=np.float32))
    values = np.ascontiguousarray(np.asarray(values, dtype=np.float32))
    queries = np.asarray(queries, dtype=np.float32)
    mask = np.asarray(attention_mask)
    w_out = np.asarray(w_out, dtype=np.float32)

    # w_host[j, p, cc, o] = w_out[j*128+o, cc*128+p]; shared by all cores
    w_host = np.ascontiguousarray(
        w_out.reshape(OC, D, H, D).transpose(0, 3, 2, 1))
    ones = np.ones((D, D), dtype=np.float32)

    in_maps = []
    for core in range(N_CORES):
        b, qh = core // 2, core % 2
        kb = keys[b, :, 0, :]                                   # [C, S]
        qb = np.ascontiguousarray(
            queries[b, :, 0, qh * QB:(qh + 1) * QB]) * SCALE    # [C, QB]
        # v_host[h, p, c, d] = values[b, h*128+d, 0, c*128+p]
        vb = np.ascontiguousarray(
            values[b, :, 0, :].reshape(H, D, KC, D).transpose(0, 3, 2, 1))
        bias = np.where(mask[b], np.float32(0.0), MASK_BIAS).astype(np.float32)
        bias = np.ascontiguousarray(bias.reshape(KC, D).T)      # [D, KC]
        in_maps.append({
            "k_in": kb, "q_in": qb, "v_in": vb,
            "w_in": w_host, "bias_in": bias, "ones_in": ones,
        })
    return in_maps


def kernel(keys, values, queries, attention_mask, w_out):
    from concourse.bass_utils import run_bass_kernel_spmd

    nc = build_nc(repeat=1)
    in_maps = shard_inputs(keys, values, queries, attention_mask, w_out)
    res = run_bass_kernel_spmd(nc, in_maps, list(range(N_CORES)))

    out = np.empty((B, C, 1, S), dtype=np.float32)
    for core in range(N_CORES):
        b, qh = core // 2, core % 2
        out[b, :, 0, qh * QB:(qh + 1) * QB] = res.results[core]["out"]
    return out


# revision 8
# speedup vs baseline: 2.1468x; 2.1468x over previous
"""Trainium2 Bass kernel for nn_AttentionTorch_77833397338547.

Computation (per batch b):
  K = keys[b,:,0,:]      [C=2048, S=1024]   (C = 16 heads x 128 head_dim)
  per head h (rows h*128:(h+1)*128 of the channel dim):
    scores[k, q] = (1/sqrt(128)) * K_h^T @ Q_h          [1024, 1024]
    P = softmax_k(scores + mask_bias)
    hid_h[d, q]  = V_h @ P                              [128, 1024]
  out[o, q] = sum_c w_out[o, c] * hid[c, q]             [2048, 1024]

Sharding: 8 cores = (batch b in 0..3) x (query half qh in 0..1).
Each core computes the full attention + out_proj for its (b, q-slice).
No cross-core communication is needed because out_proj only mixes
channels, which stay local to a core.

All matmuls run as float32r (single-pass fp32 on the PE, ~1.3e-4 rel
accuracy, 4x the throughput of plain fp32). The softmax denominator is
chunk-summed on the VectorEngine (7 adds/head) so only one ones-matmul
per head remains for the cross-partition sum + broadcast — replacing
128 PE matmuls bought ~1.5x wall time at identical accuracy.
"""

import sys

sys.path.insert(0, "/opt/trn_rl_repo")

import numpy as np

B, C, S = 4, 2048, 1024
H, D = 16, 128          # heads x head_dim
QB = S // 2             # per-core query block = 512
KC = S // D             # key chunks per head = 8
OC = C // D             # out_proj row chunks = 16
N_CORES = 8
SCALE = 1.0 / np.sqrt(np.float32(D))
MASK_BIAS = np.float32(-60.0)

_BUILT = {}

# fast=True streams E/V/w_out/hid as bf16 (2-byte moving operands → ~2x PE
# stream rate on AV/denominator/out_proj matmuls + half their DMA); QK and
# all accumulation stay fp32-class. Default False = validated f32r config.
FAST = False

# DVD=True sums the 8 E-chunks on the VectorEngine (7 adds/head) and keeps
# only ONE ones-matmul per head for the partition-sum+broadcast, replacing
# 128 PE matmuls (~107us at measured rates) with DVE work that overlaps.
DVD = True


def build_nc(repeat: int = 1, fast: bool = FAST, dvd: bool = DVD):
    """Build + compile the per-core Bass program. Cached per config."""
    key = (repeat, fast, dvd)
    if key in _BUILT:
        return _BUILT[key]

    import concourse.bass as bass
    import concourse.mybir as mybir
    import concourse.tile as tile
    from concourse import bacc

    f32 = mybir.dt.float32
    f32r = mybir.dt.float32r
    bf16 = mybir.dt.bfloat16
    edt = bf16 if fast else f32r   # E, ones, V (attention value path)
    wdt = bf16 if fast else f32r   # w_out, hid (projection path)
    EXP = mybir.ActivationFunctionType.Exp

    nc = bacc.Bacc("TRN2", target_bir_lowering=False, debug=False,
                   num_devices=N_CORES)

    k_d = nc.dram_tensor("k_in", [C, S], f32r, kind="ExternalInput")
    q_d = nc.dram_tensor("q_in", [C, QB], f32r, kind="ExternalInput")
    v_d = nc.dram_tensor("v_in", [H, D, KC, D], edt, kind="ExternalInput")
    w_d = nc.dram_tensor("w_in", [OC, D, H, D], wdt, kind="ExternalInput")
    bias_d = nc.dram_tensor("bias_in", [D, KC], f32, kind="ExternalInput")
    ones_d = nc.dram_tensor("ones_in", [D, D], edt, kind="ExternalInput")
    out_d = nc.dram_tensor("out", [C, QB], f32, kind="ExternalOutput")

    def body(tc):
        with (
            tc.tile_pool(name="const", bufs=1) as const,
            tc.tile_pool(name="kvq", bufs=2) as kvq,
            tc.tile_pool(name="ep", bufs=2) as ep,
            tc.tile_pool(name="hidp", bufs=1) as hidp,
            tc.tile_pool(name="wp", bufs=3) as wp,
            tc.tile_pool(name="rcp", bufs=2) as rcp,
            tc.tile_pool(name="osb", bufs=3) as osb,
            tc.tile_pool(name="scp", bufs=3, space="PSUM") as scp,
            tc.tile_pool(name="hpp", bufs=2, space="PSUM") as hpp,
            tc.tile_pool(name="dnp", bufs=1, space="PSUM") as dnp,
            tc.tile_pool(name="opp", bufs=2, space="PSUM") as opp,
        ):
            ones_sb = const.tile([D, D], edt)
            bias_sb = const.tile([D, KC], f32)
            nc.sync.dma_start(ones_sb[:], ones_d[:])
            nc.sync.dma_start(bias_sb[:], bias_d[:])

            hid_all = hidp.tile([D, H, QB], wdt)

            for h in range(H):
                k_sb = kvq.tile([D, S], f32r)
                q_sb = kvq.tile([D, QB], f32r)
                v_sb = kvq.tile([D, KC, D], edt)
                nc.sync.dma_start(k_sb[:], k_d[h * D:(h + 1) * D, :])
                nc.sync.dma_start(q_sb[:], q_d[h * D:(h + 1) * D, :])
                nc.sync.dma_start(v_sb[:], v_d[h])

                e_sb = ep.tile([D, KC, QB], edt)
                for c in range(KC):
                    sc = scp.tile([D, QB], f32)
                    nc.tensor.matmul(sc[:], k_sb[:, c * D:(c + 1) * D], q_sb[:],
                                     start=True, stop=True)
                    nc.scalar.activation(e_sb[:, c, :], sc[:], EXP,
                                         bias=bias_sb[:, c:c + 1], scale=1.0)

                dn = dnp.tile([D, QB], f32)
                if dvd:
                    acc = ep.tile([D, QB], edt, tag="dv")
                    nc.vector.tensor_add(acc[:], e_sb[:, 0, :], e_sb[:, 1, :])
                    for c in range(2, KC):
                        nxt = ep.tile([D, QB], edt, tag="dv")
                        nc.vector.tensor_add(nxt[:], acc[:], e_sb[:, c, :])
                        acc = nxt
                    nc.tensor.matmul(dn[:], ones_sb[:], acc[:],
                                     start=True, stop=True)
                else:
                    for c in range(KC):
                        nc.tensor.matmul(dn[:], ones_sb[:], e_sb[:, c, :],
                                         start=(c == 0), stop=(c == KC - 1))
                hp = hpp.tile([D, QB], f32)
                for c in range(KC):
                    nc.tensor.matmul(hp[:], v_sb[:, c, :], e_sb[:, c, :],
                                     start=(c == 0), stop=(c == KC - 1))

                rc = rcp.tile([D, QB], f32)
                nc.vector.reciprocal(rc[:], dn[:])
                nc.vector.tensor_mul(hid_all[:, h, :], hp[:], rc[:])

            for j in range(OC):
                w_sb = wp.tile([D, H, D], wdt)
                nc.sync.dma_start(w_sb[:], w_d[j])
                op = opp.tile([D, QB], f32)
                for cc in range(H):
                    nc.tensor.matmul(op[:], w_sb[:, cc, :], hid_all[:, cc, :],
                                     start=(cc == 0), stop=(cc == H - 1))
                o_sb = osb.tile([D, QB], f32)
                nc.vector.tensor_copy(o_sb[:], op[:])
                nc.sync.dma_start(out_d[j * D:(j + 1) * D, :], o_sb[:])

    with tile.TileContext(nc) as tc:
        if repeat == 1:
            body(tc)
        else:
            PE = mybir.EngineType.PE
            ACT = mybir.EngineType.Activation
            DVE = mybir.EngineType.DVE
            SP = mybir.EngineType.SP
            with tc.For_i(0, repeat, 1, hint_engines=(PE, ACT, DVE, SP)):
                body(tc)

    nc.compile()
    _BUILT[key] = nc
    return nc


def shard_inputs(keys, values, queries, attention_mask, w_out, fast=None):
    """Host-side prep: slice per core and pre-layout for the device."""
    if fast is None:
        fast = FAST
    if fast:
        import ml_dtypes
        vdt = wdt = ml_dtypes.bfloat16
    else:
        vdt = wdt = np.float32
    keys = np.ascontiguousarray(np.asarray(keys, dtype=np.float32))
    values = np.ascontiguousarray(np.asarray(values, dtype=np.float32))
    queries = np.asarray(queries, dtype=np.float32)
    mask = np.asarray(attention_mask)
    w_out = np.asarray(w_out, dtype=np.float32)

    # w_host[j, p, cc, o] = w_out[j*128+o, cc*128+p]; shared by all cores
    w_host = np.ascontiguousarray(
        w_out.reshape(OC, D, H, D).transpose(0, 3, 2, 1)).astype(wdt)
    ones = np.ones((D, D), dtype=vdt)

    in_maps = []
    for core in range(N_CORES):
        b, qh = core // 2, core % 2
        kb = keys[b, :, 0, :]                                   # [C, S]
        qb = np.ascontiguousarray(
            queries[b, :, 0, qh * QB:(qh + 1) * QB]) * SCALE    # [C, QB]
        # v_host[h, p, c, d] = values[b, h*128+d, 0, c*128+p]
        vb = np.ascontiguousarray(
            values[b, :, 0, :].reshape(H, D, KC, D).transpose(0, 3, 2, 1)
        ).astype(vdt)
        bias = np.where(mask[b], np.float32(0.0), MASK_BIAS).astype(np.float32)
        bias = np.ascontiguousarray(bias.reshape(KC, D).T)      # [D, KC]
        in_maps.append({
            "k_in": kb, "q_in": qb, "v_in": vb,
            "w_in": w_host, "bias_in": bias, "ones_in": ones,
        })
    return in_maps


def kernel(keys, values, queries, attention_mask, w_out):
    from concourse.bass_utils import run_bass_kernel_spmd

    nc = build_nc(repeat=1, fast=FAST, dvd=DVD)
    in_maps = shard_inputs(keys, values, queries, attention_mask, w_out,
                           fast=FAST)
    res = run_bass_kernel_spmd(nc, in_maps, list(range(N_CORES)))

    out = np.empty((B, C, 1, S), dtype=np.float32)
    for core in range(N_CORES):
        b, qh = core // 2, core % 2
        out[b, :, 0, qh * QB:(qh + 1) * QB] = res.results[core]["out"]
    return out
